# revision 6
# baseline (speedup 1.0000x reference)
"""Attention-pooling kernel for Trainium2 (8 NeuronCores, SPMD data-parallel).

Computes, for x: [B, S, H] and w: [H, 1]:
    scores[b, s] = sum_h tanh(x[b, s, h]) * w[h]
    attn = softmax(scores, axis=s)
    out[b, h]   = sum_s attn[b, s] * x[b, s, h]

Sharding: data-parallel over batch B across 8 cores (32 batches/core),
w replicated. No inter-core communication; host concatenates the shards.

Memory-regime roofline: each core reads 64 MiB of x once (~200-207 us at
the ~327-336 GB/s-while-busy HBM share measured on this part), so total =
front ramp (~2.5 us inside the exec window) + DMA stream + drain tail.
This version attacks the drain tail (measured 24 us on the previous
schedule) three ways:

  1. Every batch load is TWO 1 MiB half-DMAs. tanh_a(b) only waits on
     the first half's semaphore, so ACT starts ~3 us earlier relative
     to the load stream, and every downstream stage shifts with it.
  2. exp(b) is emitted in the ACT stream right after tanh_b(b+1) (one
     iteration earlier than the old head-of-(b+2) placement). scores(b)
     are a full iteration old at that point, so ACT does not stall, and
     the PE context-matmul train(b) now runs in iteration b+1 instead
     of b+2. The PE backlog entering the drain shrinks from ~3 trains
     (~13 us serialized after the last load) to ~1.5.
  3. Drain fine-graining: the last batch is processed at QUARTER
     granularity (load/tanh/chain/exp/ctx-matmuls per 8 s-tiles), and
     the last pair's outputs ship as two independent DMAs, so the
     post-last-load critical path is quarter-sized ops, not full-batch
     ones.

Per-core dataflow (per batch b), s-tile t in [0, 32), s = p*32 + t:
  DMA   : x[b] -> SBUF slots [1:33] of a 33-slot tile (16 KB contiguous
          per partition; float32r view of the same bytes). Slot 0 is
          never written — see the matmul trick below.
  ACT   : tanh(x) -> energy in FP16 (fp16 keeps the DVE 16-bit 2x rate
          of bf16 but with 8x the mantissa — scores |.|<40 fit easily)
  DVE   : energy *= w (fp16, in place, 2x_1p), then the h-reduction as
          an fp16 TT add-tree (128->64->32->16) + one fp32 tensor_reduce
          over the last 16. A monolithic tensor_reduce has no DVE perf
          mode and costs 2x more. All score compute stays on DVE:
          any concurrent GPSIMD op grabs the shared SBUF port pair and
          fully blocks DVE 16-bit TTs (measured: a 0.9 us mul stretched
          to 4.4 us ending exactly at GPSIMD-op end).
  ACT   : ebuf = exp(scores) (float32r), accum_out -> rowsum [128, 1]
  PE    : context via fp32r M=1 matmuls (fast path needs moving free
          >= 256), ALL accumulating into ONE psum tile ps[0, 0:256]:
          matmul for s-tile t uses lhsT=ebuf[:, t] and rhs = xb slots
          (t, t+1) (s-tiles (t-1, t) — slot 0 holds junk), so the
          useful product e_t*x_t always lands in ps[0, 128:256] and the
          garbage e_t*x_{t-1} in ps[0, 0:128]. No cross-bank add needed.
  PE    : total = rowsum.T @ ones. Regular pairs write one [1, 2] psum
          tile so a single DVE reciprocal serves the pair (halves the
          per-batch recip fixed cost); the LAST pair uses two separate
          [1, 1] tiles + two reciprocals so out(30) ships early and
          out(31)'s recip doesn't wait on anything pair-shaped.
  ACT   : out_row = ps[0,128:256] * recip; DMA out on the scalar ring.

Pipelining: consumers are deferred so every engine only waits on work
from previous iterations. Iteration i emits:
  DMA   load(i) in halves (quarters for the last batch);
  ACT   tanh_a(i), tanh_b(i), exp(i-1), [pair copies + out-DMA];
  DVE   [pair recip], mul/tree/reduce chain(i) (mul split at the tanh
        half boundary — decoupling is load-bearing: a fused mul re-forms
        the ACT->DVE serial cycle, measured +38 us end-to-end);
  PE    tot(i-1) + 33 ctx matmuls(i-1).
Softmax normalization is factored out of the weighted sum (exp without
max-subtraction is safe: |scores| < ~40 here).
"""

import numpy as np

import concourse.bass as bass
import concourse.tile as tile
from concourse import bacc, mybir
from concourse.bass_utils import run_bass_kernel_spmd

B, S, H = 256, 4096, 128
N_CORES = 8
B_SHARD = B // N_CORES  # 32
P = 128                 # SBUF partitions; also H
S_TILES = S // P        # 32  (s = p * S_TILES + t)
XSLOTS = S_TILES + 1    # slot 0 = junk pad for the shifted-pair matmul
LAST = B_SHARD - 1

F32 = mybir.dt.float32
F32R = mybir.dt.float32r
F16 = mybir.dt.float16

_nc_cache = None


def _build() -> bass.Bass:
    nc = bacc.Bacc(None, target_bir_lowering=False, enable_partition_id=False)

    x_ext = nc.declare_dram_parameter(
        "encoder_outputs", [B_SHARD, S, H], F32, isOutput=False
    )
    w_ext = nc.declare_dram_parameter(
        "attention_weights", [H, 1], F32, isOutput=False
    )
    out_ext = nc.declare_dram_parameter("out", [B_SHARD, H], F32, isOutput=True)

    with tile.TileContext(nc) as tc:
        with (
            tc.tile_pool(name="singles", bufs=1) as singles,
            tc.tile_pool(name="xpool", bufs=9) as xpool,
            tc.tile_pool(name="evpool", bufs=3) as evpool,
            tc.tile_pool(name="small", bufs=8) as small,
            tc.tile_pool(name="psum_ctx", bufs=4, space="PSUM") as psum_ctx_pool,
            tc.tile_pool(name="psum_tot", bufs=2, space="PSUM") as psum_tot_pool,
            tc.tile_pool(name="psum_w", bufs=1, space="PSUM") as psum_w_pool,
        ):
            # w arrives as a plain [1, H] row (one descriptor, ~1.5 us);
            # the partition broadcast is a one-shot K=1 PE matmul
            # out[m, n] = ones[0, m] * w[0, n] into PSUM. The previous
            # partition-stride-0 broadcast DMA (DRE replicate) measured
            # ~7.8 us and gated the whole startup.
            w0 = singles.tile([1, H], F32)
            w_flat = w_ext[:].rearrange("h one -> (one h)")
            w_row = bass.AP(
                tensor=w_flat.tensor,
                offset=w_flat.offset,
                ap=[[0, 1], w_flat.ap[0]],
            )
            nc.scalar.dma_start(out=w0, in_=w_row)

            ones_row = singles.tile([1, H], F32)
            nc.vector.memset(ones_row, 1.0)
            wb_ps = psum_w_pool.tile([P, H], F32)
            nc.tensor.matmul(wb_ps, ones_row, w0, start=True, stop=True)

            ones_col = singles.tile([P, 1], F32)
            nc.vector.memset(ones_col, 1.0)

            # w replicated along the tile axis in fp16 (DVE is the only
            # reader). Log-doubling: 6 copies instead of 32 so the fill
            # phase isn't serialized behind ~8 us of setup casts.
            w_rep = singles.tile([P, S_TILES, H], F16)
            nc.vector.tensor_copy(w_rep[:, 0, :], wb_ps)
            n = 1
            while n < S_TILES:
                m = min(n, S_TILES - n)
                nc.vector.tensor_copy(
                    w_rep[:, n : n + m, :], w_rep[:, 0:m, :]
                )
                n += m

            # [b, p, t, h] view of DRAM; partition p reads 16 KB contiguous.
            # (Pair-granularity 4.3 MB loads were tried and regressed:
            # with pair-sized ring slots the 4-deep ring can't cover the
            # fill latency and the DMA front stalls ~40 us.)
            xv = x_ext[:].rearrange("b (p t) h -> b p t h", p=P)

            st = [dict() for _ in range(B_SHARD)]

            HALF = S_TILES // 2
            QUART = S_TILES // 4

            def stage0(b, d):  # load into slots [1:33]; slot 0 stays junk
                d["xb"] = xb = xpool.tile([P, XSLOTS, H], F32R, tag="xb", name="xb")
                xvb = xv[b].bitcast(F32R)
                # Halved loads: tanh_a(b) waits only the first 1 MiB's
                # semaphore. The last batch loads in quarters so the
                # drain's tanh/chain quarters fire as early as possible.
                step = QUART if b == LAST else HALF
                for lo in range(0, S_TILES, step):
                    nc.sync.dma_start(
                        out=xb[:, 1 + lo : 1 + lo + step, :],
                        in_=xvb[:, lo : lo + step, :],
                    )

            def tanh_chunk(b, d, lo, hi):
                if "ev" not in d:
                    d["ev"] = evpool.tile([P, S_TILES, H], F16, tag="ev", name="ev")
                xbf = d["xb"].bitcast(F32)
                nc.scalar.activation(
                    out=d["ev"][:, lo:hi, :],
                    in_=xbf[:, 1 + lo : 1 + hi, :],
                    func=mybir.ActivationFunctionType.Tanh,
                )

            def stage1(b, d):  # tanh -> fp16 energy (halves; last: quarters)
                step = QUART if b == LAST else HALF
                for lo in range(0, S_TILES, step):
                    tanh_chunk(b, d, lo, lo + step)

            def stage_chain(b, d):  # DVE: mul + fp16 tree + fp32 reduce
                # The mul is split at the tanh half boundary so DVE starts
                # on mul_a as soon as tanh_a lands instead of waiting for
                # the full tanh. This decoupling is load-bearing: a single
                # fused mul re-forms the ACT->DVE serial cycle and costs
                # ~38 us end-to-end (measured 246 us vs 207 us).
                d["scores"] = small.tile([P, S_TILES], F32, tag="scores", name="scores")
                ev = d["ev"]
                if b >= LAST - 1:
                    # Drain batches: fully chunk-granular chain so each
                    # chunk's scores (and exp, and PE matmuls) are ready
                    # as soon as its tanh chunk lands.
                    step = QUART if b == LAST else HALF
                    for lo in range(0, S_TILES, step):
                        hi = lo + step
                        sl = ev[:, lo:hi, :]
                        nc.vector.tensor_mul(sl, sl, w_rep[:, lo:hi, :])
                        nc.vector.tensor_add(
                            sl[:, :, 0:64], sl[:, :, 0:64], sl[:, :, 64:128]
                        )
                        nc.vector.tensor_add(
                            sl[:, :, 0:32], sl[:, :, 0:32], sl[:, :, 32:64]
                        )
                        nc.vector.tensor_add(
                            sl[:, :, 0:16], sl[:, :, 0:16], sl[:, :, 16:32]
                        )
                        nc.vector.tensor_reduce(
                            out=d["scores"][:, lo:hi],
                            in_=sl[:, :, 0:16],
                            axis=mybir.AxisListType.X,
                            op=mybir.AluOpType.add,
                        )
                    return
                nc.vector.tensor_mul(
                    ev[:, 0:HALF, :], ev[:, 0:HALF, :], w_rep[:, 0:HALF, :]
                )
                nc.vector.tensor_mul(
                    ev[:, HALF:, :], ev[:, HALF:, :], w_rep[:, HALF:, :]
                )
                nc.vector.tensor_add(ev[:, :, 0:64], ev[:, :, 0:64], ev[:, :, 64:128])
                nc.vector.tensor_add(ev[:, :, 0:32], ev[:, :, 0:32], ev[:, :, 32:64])
                nc.vector.tensor_add(ev[:, :, 0:16], ev[:, :, 0:16], ev[:, :, 16:32])
                nc.vector.tensor_reduce(
                    out=d["scores"],
                    in_=ev[:, :, 0:16],
                    axis=mybir.AxisListType.X,
                    op=mybir.AluOpType.add,
                )

            def exp_chunk(b, d, k, lo, hi):
                if "ebuf" not in d:
                    d["ebuf"] = small.tile(
                        [P, S_TILES], F32R, tag="ebuf", name="ebuf"
                    )
                    d["rowsums"] = []
                r = small.tile([P, 1], F32, tag=f"rowsum_{k}", name=f"rowsum_{k}")
                d["rowsums"].append(r)
                nc.scalar.activation(
                    out=d["ebuf"][:, lo:hi],
                    in_=d["scores"][:, lo:hi],
                    func=mybir.ActivationFunctionType.Exp,
                    accum_out=r,
                )

            def stage_exp(b, d):  # exp(scores) -> ebuf, rowsum chunks (ACT)
                if b >= LAST - 1:
                    step = QUART if b == LAST else HALF
                    for k, lo in enumerate(range(0, S_TILES, step)):
                        exp_chunk(b, d, k, lo, lo + step)
                    return
                exp_chunk(b, d, 0, 0, S_TILES)

            def stage4(b, d):  # fp32r shifted-pair matmuls, one psum bank
                xb, ebuf = d["xb"], d["ebuf"]
                # Regular pairs: both batches write one [1, 2] psum tile so
                # a single reciprocal serves the pair. The last pair gets
                # separate [1, 1] tiles so each reciprocal/out ships alone.
                # The tot matmul goes FIRST in the block: its rowsum input
                # is ready with exp, and the reciprocal (first thing DVE
                # wants next iteration) otherwise waits for the tail of
                # this 33-matmul block.
                c = b % 2
                if b >= LAST - 1:
                    tot1 = psum_tot_pool.tile([1, 1], F32, tag="tot2", name="tot1")
                    d["tot1"] = tot1
                    step = QUART if b == LAST else HALF
                    nsegs = S_TILES // step
                    rs = d["rowsums"]
                    ps = psum_ctx_pool.tile([1, 2 * H], F32, tag="ps", name="ps")
                    d["ps"] = ps
                    for k in range(nsegs):
                        nc.tensor.matmul(
                            tot1, rs[k], ones_col,
                            start=(k == 0), stop=(k == nsegs - 1),
                        )
                        for t in range(k * step, (k + 1) * step):
                            nc.tensor.matmul(
                                ps,
                                ebuf[:, t : t + 1],
                                xb[:, t : t + 2, :],
                                start=(t == 0),
                                stop=(t == S_TILES - 1),
                            )
                    return
                if c == 0:
                    tot2 = psum_tot_pool.tile([1, 2], F32, tag="tot2", name="tot2")
                    d["tot2"] = tot2
                else:
                    tot2 = st[b - 1]["tot2"]
                nc.tensor.matmul(
                    tot2[0:1, c : c + 1], d["rowsums"][0], ones_col,
                    start=True, stop=True,
                )
                ps = psum_ctx_pool.tile([1, 2 * H], F32, tag="ps", name="ps")
                for t in range(S_TILES):
                    # rhs slots (t, t+1) = s-tiles (t-1, t); useful half
                    # e_t * x_t lands in ps[0, 128:256]
                    nc.tensor.matmul(
                        ps,
                        ebuf[:, t : t + 1],
                        xb[:, t : t + 2, :],
                        start=(t == 0),
                        stop=(t == S_TILES - 1),
                    )
                d["ps"] = ps

            def stage5_dve(b, d):  # DVE: one reciprocal per regular pair
                recip2 = small.tile([1, 2], F32, tag="recip2", name="recip2")
                nc.vector.reciprocal(out=recip2, in_=st[b - 1]["tot2"])
                st[b - 1]["recip"] = recip2[0:1, 0:1]
                d["recip"] = recip2[0:1, 1:2]

            def stage5_dve_single(b, d):  # DVE: reciprocal for a drain single
                recip1 = small.tile([1, 1], F32, tag="recip2", name="recip1")
                nc.vector.reciprocal(out=recip1, in_=d["tot1"])
                d["recip"] = recip1

            # flat [B_SHARD*H] view so a pair's two rows DMA as one [1, 2H]
            out_flat = out_ext[:].rearrange("b h -> (b h)")

            def stage5_act(b, d):  # ACT: normalize + store (regular pairs)
                # Both rows of a pair land in one [1, 2H] tile and ship in
                # ONE scalar-ring DMA: each dma_start issue (+ lane-sem
                # wait) on the ACT queue blocks the compute stream, and 32
                # of them cost ~30 us of measured ACT holes.
                c = b % 2
                if c == 0:
                    d["out2"] = out2 = small.tile([1, 2 * H], F32, tag="out2", name="out2")
                else:
                    out2 = st[b - 1]["out2"]
                nc.scalar.activation(
                    out=out2[0:1, c * H : (c + 1) * H],
                    in_=d["ps"][0:1, H : 2 * H],
                    func=mybir.ActivationFunctionType.Copy,
                    scale=d["recip"],
                )
                if c == 1:
                    span = out_flat[(b - 1) * H : (b + 1) * H]
                    dst = bass.AP(
                        tensor=span.tensor, offset=span.offset,
                        ap=[[0, 1], span.ap[0]],
                    )
                    nc.scalar.dma_start(out=dst, in_=out2)

            def stage5_act_single(b, d):  # ACT: normalize + store one row
                out1 = small.tile([1, H], F32, tag="out2", name="out1")
                nc.scalar.activation(
                    out=out1,
                    in_=d["ps"][0:1, H : 2 * H],
                    func=mybir.ActivationFunctionType.Copy,
                    scale=d["recip"],
                )
                span = out_flat[b * H : (b + 1) * H]
                dst = bass.AP(
                    tensor=span.tensor, offset=span.offset,
                    ap=[[0, 1], span.ap[0]],
                )
                nc.scalar.dma_start(out=dst, in_=out1)

            def live(j):
                return 0 <= j < B_SHARD

            for i in range(B_SHARD + 2):
                if live(i):
                    stage0(i, st[i])
                # ACT stream: tanh(i), then exp(i-1) (scores a full
                # iteration old -> no stall), then pair copies. At the
                # drain, exp chunks are woven between tanh chunks so PE
                # never waits for a full batch of ACT work.
                if i == LAST:
                    # tanh(31) quarters interleaved with exp(30) halves
                    tanh_chunk(i, st[i], 0, QUART)
                    exp_chunk(i - 1, st[i - 1], 0, 0, HALF)
                    tanh_chunk(i, st[i], QUART, HALF)
                    exp_chunk(i - 1, st[i - 1], 1, HALF, S_TILES)
                    tanh_chunk(i, st[i], HALF, HALF + QUART)
                    tanh_chunk(i, st[i], HALF + QUART, S_TILES)
                else:
                    if live(i):
                        stage1(i, st[i])
                    if live(i - 1):
                        stage_exp(i - 1, st[i - 1])
                # DVE stream: epilogue recip first, then own score chain.
                odd = i - 2
                do_pair = live(odd) and odd % 2 == 1 and odd < LAST
                if do_pair:
                    stage5_dve(odd, st[odd])
                # Drain singles: recip(b) for b in {LAST-1, LAST} once
                # tot(b) is in psum (tot matmuls finish early in train b).
                if i == LAST + 1:
                    stage5_dve_single(LAST - 1, st[LAST - 1])
                if i == LAST + 2:
                    stage5_dve_single(LAST, st[LAST])
                if live(i):
                    stage_chain(i, st[i])
                if live(i - 1):
                    stage4(i - 1, st[i - 1])
                if do_pair:
                    stage5_act(odd - 1, st[odd - 1])
                    stage5_act(odd, st[odd])
                if i == LAST + 1:
                    stage5_act_single(LAST - 1, st[LAST - 1])
                if i == LAST + 2:
                    stage5_act_single(LAST, st[LAST])

    # Bacc pipeline: splits multi-sem waits (HW allows one per instr),
    # inserts GPSIMD library loads + ACT table loads, lowers extended ISA.
    nc.compile()
    return nc


def _get_nc() -> bass.Bass:
    global _nc_cache
    if _nc_cache is None:
        _nc_cache = _build()
    return _nc_cache


def run(encoder_outputs: np.ndarray, attention_weights: np.ndarray, **spmd_kwargs):
    """Run the SPMD kernel; returns (output [B, H], BassKernelResults)."""
    nc = _get_nc()
    x = np.ascontiguousarray(encoder_outputs, dtype=np.float32)
    w = np.ascontiguousarray(attention_weights, dtype=np.float32)
    assert x.shape == (B, S, H), x.shape
    assert w.shape == (H, 1), w.shape
    in_maps = [
        {
            "encoder_outputs": x[i * B_SHARD : (i + 1) * B_SHARD],
            "attention_weights": w,
        }
        for i in range(N_CORES)
    ]
    res = run_bass_kernel_spmd(nc, in_maps, core_ids=list(range(N_CORES)), **spmd_kwargs)
    out = np.concatenate(
        [res.results[i]["out"] for i in range(N_CORES)], axis=0
    ).astype(np.float32)
    return out, res


def kernel(encoder_outputs: np.ndarray, attention_weights: np.ndarray) -> np.ndarray:
    out, _ = run(encoder_outputs, attention_weights)
    return out


# revision 8
# speedup vs baseline: 1.0897x; 1.0897x over previous
"""Attention-pooling kernel for Trainium2 (8 NeuronCores, SPMD data-parallel).

Computes, for x: [B, S, H] and w: [H, 1]:
    scores[b, s] = sum_h tanh(x[b, s, h]) * w[h]
    attn = softmax(scores, axis=s)
    out[b, h]   = sum_s attn[b, s] * x[b, s, h]

Sharding: data-parallel over batch B across 8 cores (32 batches/core),
w replicated. No inter-core communication; host concatenates the shards.

Memory-regime roofline: each core reads 64 MiB of x once (~200-207 us at
the ~327-336 GB/s-while-busy HBM share measured on this part), so total =
front ramp (~2.5 us inside the exec window) + DMA stream + drain tail.
This version attacks the drain tail (measured 24 us on the previous
schedule) three ways:

  1. Every batch load is TWO 1 MiB half-DMAs. tanh_a(b) only waits on
     the first half's semaphore, so ACT starts ~3 us earlier relative
     to the load stream, and every downstream stage shifts with it.
  2. exp(b) is emitted in the ACT stream right after tanh_b(b+1) (one
     iteration earlier than the old head-of-(b+2) placement). scores(b)
     are a full iteration old at that point, so ACT does not stall, and
     the PE context-matmul train(b) now runs in iteration b+1 instead
     of b+2. The PE backlog entering the drain shrinks from ~3 trains
     (~13 us serialized after the last load) to ~1.5.
  3. Drain fine-graining: the last batch is processed at QUARTER
     granularity (load/tanh/chain/exp/ctx-matmuls per 8 s-tiles), and
     the last pair's outputs ship as two independent DMAs, so the
     post-last-load critical path is quarter-sized ops, not full-batch
     ones.

Per-core dataflow (per batch b), s-tile t in [0, 32), s = p*32 + t:
  DMA   : x[b] -> SBUF slots [1:33] of a 33-slot tile (16 KB contiguous
          per partition; float32r view of the same bytes). Slot 0 is
          never written — see the matmul trick below.
  ACT   : tanh(x) -> energy in FP16 (fp16 keeps the DVE 16-bit 2x rate
          of bf16 but with 8x the mantissa — scores |.|<40 fit easily)
  DVE   : energy *= w (fp16, in place, 2x_1p), then the h-reduction as
          an fp16 TT add-tree (128->64->32->16) + one fp32 tensor_reduce
          over the last 16. A monolithic tensor_reduce has no DVE perf
          mode and costs 2x more. All score compute stays on DVE:
          any concurrent GPSIMD op grabs the shared SBUF port pair and
          fully blocks DVE 16-bit TTs (measured: a 0.9 us mul stretched
          to 4.4 us ending exactly at GPSIMD-op end).
  ACT   : ebuf = exp(scores) (float32r), accum_out -> rowsum [128, 1]
  PE    : context via fp32r M=1 matmuls (fast path needs moving free
          >= 256), ALL accumulating into ONE psum tile ps[0, 0:256]:
          matmul for s-tile t uses lhsT=ebuf[:, t] and rhs = xb slots
          (t, t+1) (s-tiles (t-1, t) — slot 0 holds junk), so the
          useful product e_t*x_t always lands in ps[0, 128:256] and the
          garbage e_t*x_{t-1} in ps[0, 0:128]. No cross-bank add needed.
  PE    : total = rowsum.T @ ones. Regular pairs write one [1, 2] psum
          tile so a single DVE reciprocal serves the pair (halves the
          per-batch recip fixed cost); the LAST pair uses two separate
          [1, 1] tiles + two reciprocals so out(30) ships early and
          out(31)'s recip doesn't wait on anything pair-shaped.
  ACT   : out_row = ps[0,128:256] * recip; DMA out on the scalar ring.

Pipelining: consumers are deferred so every engine only waits on work
from previous iterations. Iteration i emits:
  DMA   load(i) in halves (quarters for the last batch);
  ACT   tanh_a(i), tanh_b(i), exp(i-1), [pair copies + out-DMA];
  DVE   [pair recip], mul/tree/reduce chain(i) (mul split at the tanh
        half boundary — decoupling is load-bearing: a fused mul re-forms
        the ACT->DVE serial cycle, measured +38 us end-to-end);
  PE    tot(i-1) + 33 ctx matmuls(i-1).
Softmax normalization is factored out of the weighted sum (exp without
max-subtraction is safe: |scores| < ~40 here).
"""

import numpy as np

import concourse.bass as bass
import concourse.tile as tile
from concourse import bacc, mybir
from concourse.bass_utils import run_bass_kernel_spmd

B, S, H = 256, 4096, 128
N_CORES = 8
B_SHARD = B // N_CORES  # 32
P = 128                 # SBUF partitions; also H
S_TILES = S // P        # 32  (s = p * S_TILES + t)
XSLOTS = S_TILES + 1    # slot 0 = junk pad for the shifted-pair matmul
LAST = B_SHARD - 1

F32 = mybir.dt.float32
F32R = mybir.dt.float32r
F16 = mybir.dt.float16

_nc_cache = None


def _build() -> bass.Bass:
    nc = bacc.Bacc(None, target_bir_lowering=False, enable_partition_id=False)

    x_ext = nc.declare_dram_parameter(
        "encoder_outputs", [B_SHARD, S, H], F32, isOutput=False
    )
    w_ext = nc.declare_dram_parameter(
        "attention_weights", [H, 1], F32, isOutput=False
    )
    out_ext = nc.declare_dram_parameter("out", [B_SHARD, H], F32, isOutput=True)

    with tile.TileContext(nc) as tc:
        with (
            tc.tile_pool(name="singles", bufs=1) as singles,
            tc.tile_pool(name="xpool", bufs=9) as xpool,
            tc.tile_pool(name="evpool", bufs=3) as evpool,
            tc.tile_pool(name="small", bufs=8) as small,
            tc.tile_pool(name="psum_ctx", bufs=4, space="PSUM") as psum_ctx_pool,
            tc.tile_pool(name="psum_tot", bufs=2, space="PSUM") as psum_tot_pool,
            tc.tile_pool(name="psum_w", bufs=1, space="PSUM") as psum_w_pool,
        ):
            # w arrives as a plain [1, H] row (one descriptor, ~1.5 us);
            # the partition broadcast is a one-shot K=1 PE matmul
            # out[m, n] = ones[0, m] * w[0, n] into PSUM. The previous
            # partition-stride-0 broadcast DMA (DRE replicate) measured
            # ~7.8 us and gated the whole startup.
            w0 = singles.tile([1, H], F32)
            w_flat = w_ext[:].rearrange("h one -> (one h)")
            w_row = bass.AP(
                tensor=w_flat.tensor,
                offset=w_flat.offset,
                ap=[[0, 1], w_flat.ap[0]],
            )
            nc.scalar.dma_start(out=w0, in_=w_row)

            ones_row = singles.tile([1, H], F32)
            nc.vector.memset(ones_row, 1.0)
            wb_ps = psum_w_pool.tile([P, H], F32)
            nc.tensor.matmul(wb_ps, ones_row, w0, start=True, stop=True)

            ones_col = singles.tile([P, 1], F32)
            nc.vector.memset(ones_col, 1.0)

            # w replicated along the tile axis in fp16 (DVE is the only
            # reader). Log-doubling: 6 copies instead of 32 so the fill
            # phase isn't serialized behind ~8 us of setup casts.
            w_rep = singles.tile([P, S_TILES, H], F16)
            nc.vector.tensor_copy(w_rep[:, 0, :], wb_ps)
            n = 1
            while n < S_TILES:
                m = min(n, S_TILES - n)
                nc.vector.tensor_copy(
                    w_rep[:, n : n + m, :], w_rep[:, 0:m, :]
                )
                n += m

            # [b, p, t, h] view of DRAM; partition p reads 16 KB contiguous.
            # (Pair-granularity 4.3 MB loads were tried and regressed:
            # with pair-sized ring slots the 4-deep ring can't cover the
            # fill latency and the DMA front stalls ~40 us.)
            xv = x_ext[:].rearrange("b (p t) h -> b p t h", p=P)

            st = [dict() for _ in range(B_SHARD)]

            HALF = S_TILES // 2
            QUART = S_TILES // 4

            def stage0(b, d):  # load into slots [1:33]; slot 0 stays junk
                d["xb"] = xb = xpool.tile([P, XSLOTS, H], F32R, tag="xb", name="xb")
                xvb = xv[b].bitcast(F32R)
                # Full 2 MiB single-DMA loads for the steady state: the
                # Tile scheduler hands HWDGE completion sems out round-robin
                # from 8 lanes, and out-store DMAs fire their sems LATE
                # (their queue row starves behind the x ring + HBM write
                # receipt). With doubled load counts a load lands on an
                # out-store's lane within ~10 us and the load ISSUE stalls
                # ~2 us every pair (measured). Keeping the DMA count at
                # ~1.6/batch makes the reuse distance ~5 batches -> no
                # stalls. Only the drain batches split: batch 0 (pipeline
                # spin-up), 29/30 in halves and 31 in quarters so the
                # drain's tanh/chain chunks start as early as possible.
                if b == LAST:
                    step = QUART
                elif b == 0 or b >= LAST - 2:
                    step = HALF
                else:
                    step = S_TILES
                for lo in range(0, S_TILES, step):
                    nc.sync.dma_start(
                        out=xb[:, 1 + lo : 1 + lo + step, :],
                        in_=xvb[:, lo : lo + step, :],
                    )

            def tanh_chunk(b, d, lo, hi):
                if "ev" not in d:
                    d["ev"] = evpool.tile([P, S_TILES, H], F16, tag="ev", name="ev")
                xbf = d["xb"].bitcast(F32)
                nc.scalar.activation(
                    out=d["ev"][:, lo:hi, :],
                    in_=xbf[:, 1 + lo : 1 + hi, :],
                    func=mybir.ActivationFunctionType.Tanh,
                )

            def stage1(b, d):  # tanh -> fp16 energy (halves; last: quarters)
                step = QUART if b == LAST else HALF
                for lo in range(0, S_TILES, step):
                    tanh_chunk(b, d, lo, lo + step)

            def stage_chain(b, d):  # DVE: mul + fp16 tree + fp32 reduce
                # The mul is split at the tanh half boundary so DVE starts
                # on mul_a as soon as tanh_a lands instead of waiting for
                # the full tanh. This decoupling is load-bearing: a single
                # fused mul re-forms the ACT->DVE serial cycle and costs
                # ~38 us end-to-end (measured 246 us vs 207 us).
                d["scores"] = small.tile([P, S_TILES], F32, tag="scores", name="scores")
                ev = d["ev"]
                if b >= LAST - 1:
                    # Drain batches: fully chunk-granular chain so each
                    # chunk's scores (and exp, and PE matmuls) are ready
                    # as soon as its tanh chunk lands.
                    step = QUART if b == LAST else HALF
                    for lo in range(0, S_TILES, step):
                        hi = lo + step
                        sl = ev[:, lo:hi, :]
                        nc.vector.tensor_mul(sl, sl, w_rep[:, lo:hi, :])
                        nc.vector.tensor_add(
                            sl[:, :, 0:64], sl[:, :, 0:64], sl[:, :, 64:128]
                        )
                        nc.vector.tensor_add(
                            sl[:, :, 0:32], sl[:, :, 0:32], sl[:, :, 32:64]
                        )
                        nc.vector.tensor_add(
                            sl[:, :, 0:16], sl[:, :, 0:16], sl[:, :, 16:32]
                        )
                        nc.vector.tensor_reduce(
                            out=d["scores"][:, lo:hi],
                            in_=sl[:, :, 0:16],
                            axis=mybir.AxisListType.X,
                            op=mybir.AluOpType.add,
                        )
                    return
                nc.vector.tensor_mul(
                    ev[:, 0:HALF, :], ev[:, 0:HALF, :], w_rep[:, 0:HALF, :]
                )
                nc.vector.tensor_mul(
                    ev[:, HALF:, :], ev[:, HALF:, :], w_rep[:, HALF:, :]
                )
                nc.vector.tensor_add(ev[:, :, 0:64], ev[:, :, 0:64], ev[:, :, 64:128])
                nc.vector.tensor_add(ev[:, :, 0:32], ev[:, :, 0:32], ev[:, :, 32:64])
                nc.vector.tensor_add(ev[:, :, 0:16], ev[:, :, 0:16], ev[:, :, 16:32])
                nc.vector.tensor_reduce(
                    out=d["scores"],
                    in_=ev[:, :, 0:16],
                    axis=mybir.AxisListType.X,
                    op=mybir.AluOpType.add,
                )

            def exp_chunk(b, d, k, lo, hi):
                if "ebuf" not in d:
                    d["ebuf"] = small.tile(
                        [P, S_TILES], F32R, tag="ebuf", name="ebuf"
                    )
                    d["rowsums"] = []
                r = small.tile([P, 1], F32, tag=f"rowsum_{k}", name=f"rowsum_{k}")
                d["rowsums"].append(r)
                nc.scalar.activation(
                    out=d["ebuf"][:, lo:hi],
                    in_=d["scores"][:, lo:hi],
                    func=mybir.ActivationFunctionType.Exp,
                    accum_out=r,
                )

            def stage_exp(b, d):  # exp(scores) -> ebuf, rowsum chunks (ACT)
                if b >= LAST - 1:
                    step = QUART if b == LAST else HALF
                    for k, lo in enumerate(range(0, S_TILES, step)):
                        exp_chunk(b, d, k, lo, lo + step)
                    return
                exp_chunk(b, d, 0, 0, S_TILES)

            def stage4(b, d):  # fp32r shifted-pair matmuls, one psum bank
                xb, ebuf = d["xb"], d["ebuf"]
                # Regular pairs: both batches write one [1, 2] psum tile so
                # a single reciprocal serves the pair. The last pair gets
                # separate [1, 1] tiles so each reciprocal/out ships alone.
                # The tot matmul goes FIRST in the block: its rowsum input
                # is ready with exp, and the reciprocal (first thing DVE
                # wants next iteration) otherwise waits for the tail of
                # this 33-matmul block.
                c = b % 2
                if b >= LAST - 1:
                    tot1 = psum_tot_pool.tile([1, 1], F32, tag="tot2", name="tot1")
                    d["tot1"] = tot1
                    step = QUART if b == LAST else HALF
                    nsegs = S_TILES // step
                    rs = d["rowsums"]
                    ps = psum_ctx_pool.tile([1, 2 * H], F32, tag="ps", name="ps")
                    d["ps"] = ps
                    for k in range(nsegs):
                        nc.tensor.matmul(
                            tot1, rs[k], ones_col,
                            start=(k == 0), stop=(k == nsegs - 1),
                        )
                        for t in range(k * step, (k + 1) * step):
                            nc.tensor.matmul(
                                ps,
                                ebuf[:, t : t + 1],
                                xb[:, t : t + 2, :],
                                start=(t == 0),
                                stop=(t == S_TILES - 1),
                            )
                    return
                if c == 0:
                    tot2 = psum_tot_pool.tile([1, 2], F32, tag="tot2", name="tot2")
                    d["tot2"] = tot2
                else:
                    tot2 = st[b - 1]["tot2"]
                nc.tensor.matmul(
                    tot2[0:1, c : c + 1], d["rowsums"][0], ones_col,
                    start=True, stop=True,
                )
                ps = psum_ctx_pool.tile([1, 2 * H], F32, tag="ps", name="ps")
                for t in range(S_TILES):
                    # rhs slots (t, t+1) = s-tiles (t-1, t); useful half
                    # e_t * x_t lands in ps[0, 128:256]
                    nc.tensor.matmul(
                        ps,
                        ebuf[:, t : t + 1],
                        xb[:, t : t + 2, :],
                        start=(t == 0),
                        stop=(t == S_TILES - 1),
                    )
                d["ps"] = ps

            def stage5_dve(b, d):  # DVE: one reciprocal per regular pair
                recip2 = small.tile([1, 2], F32, tag="recip2", name="recip2")
                nc.vector.reciprocal(out=recip2, in_=st[b - 1]["tot2"])
                st[b - 1]["recip"] = recip2[0:1, 0:1]
                d["recip"] = recip2[0:1, 1:2]

            def stage5_dve_single(b, d):  # DVE: reciprocal for a drain single
                recip1 = small.tile([1, 1], F32, tag="recip2", name="recip1")
                nc.vector.reciprocal(out=recip1, in_=d["tot1"])
                d["recip"] = recip1

            # flat [B_SHARD*H] view so a pair's two rows DMA as one [1, 2H]
            out_flat = out_ext[:].rearrange("b h -> (b h)")

            def stage5_act(b, d):  # ACT: normalize + store (regular pairs)
                # Both rows of a pair land in one [1, 2H] tile and ship in
                # ONE scalar-ring DMA: each dma_start issue (+ lane-sem
                # wait) on the ACT queue blocks the compute stream, and 32
                # of them cost ~30 us of measured ACT holes.
                c = b % 2
                if c == 0:
                    d["out2"] = out2 = small.tile([1, 2 * H], F32, tag="out2", name="out2")
                else:
                    out2 = st[b - 1]["out2"]
                nc.scalar.activation(
                    out=out2[0:1, c * H : (c + 1) * H],
                    in_=d["ps"][0:1, H : 2 * H],
                    func=mybir.ActivationFunctionType.Copy,
                    scale=d["recip"],
                )
                if c == 1:
                    span = out_flat[(b - 1) * H : (b + 1) * H]
                    dst = bass.AP(
                        tensor=span.tensor, offset=span.offset,
                        ap=[[0, 1], span.ap[0]],
                    )
                    nc.scalar.dma_start(out=dst, in_=out2)

            def stage5_act_single(b, d):  # ACT: normalize + store one row
                out1 = small.tile([1, H], F32, tag="out2", name="out1")
                nc.scalar.activation(
                    out=out1,
                    in_=d["ps"][0:1, H : 2 * H],
                    func=mybir.ActivationFunctionType.Copy,
                    scale=d["recip"],
                )
                span = out_flat[b * H : (b + 1) * H]
                dst = bass.AP(
                    tensor=span.tensor, offset=span.offset,
                    ap=[[0, 1], span.ap[0]],
                )
                nc.scalar.dma_start(out=dst, in_=out1)

            def live(j):
                return 0 <= j < B_SHARD

            for i in range(B_SHARD + 2):
                if live(i):
                    stage0(i, st[i])
                # ACT stream: tanh(i), then exp(i-1) (scores a full
                # iteration old -> no stall), then pair copies. At the
                # drain, exp chunks are woven between tanh chunks so PE
                # never waits for a full batch of ACT work.
                if i == LAST:
                    # tanh(31) quarters interleaved with exp(30) halves
                    tanh_chunk(i, st[i], 0, QUART)
                    exp_chunk(i - 1, st[i - 1], 0, 0, HALF)
                    tanh_chunk(i, st[i], QUART, HALF)
                    exp_chunk(i - 1, st[i - 1], 1, HALF, S_TILES)
                    tanh_chunk(i, st[i], HALF, HALF + QUART)
                    tanh_chunk(i, st[i], HALF + QUART, S_TILES)
                else:
                    if live(i):
                        stage1(i, st[i])
                    if live(i - 1):
                        stage_exp(i - 1, st[i - 1])
                # DVE stream: epilogue recip first, then own score chain.
                # The recip for pair (2k, 2k+1) runs at iteration 2k+3 (tot
                # matmuls sit at the head of train 2k+1, iteration 2k+2);
                # the ACT copies run one iteration LATER (2k+4) so they
                # never block the ACT stream waiting on the PE train
                # (measured 2.5 us ACT stalls when copies chase the train).
                odd = i - 2
                if live(odd) and odd % 2 == 1 and odd < LAST:
                    stage5_dve(odd, st[odd])
                # Drain singles: recip(b) for b in {LAST-1, LAST} once
                # tot(b) is in psum (tot matmuls finish early in train b).
                if i == LAST + 1:
                    stage5_dve_single(LAST - 1, st[LAST - 1])
                if i == LAST + 2:
                    stage5_dve_single(LAST, st[LAST])
                if live(i):
                    stage_chain(i, st[i])
                if live(i - 1):
                    stage4(i - 1, st[i - 1])
                odd3 = i - 3
                if live(odd3) and odd3 % 2 == 1 and odd3 < LAST:
                    stage5_act(odd3 - 1, st[odd3 - 1])
                    stage5_act(odd3, st[odd3])
                if i == LAST + 1:
                    stage5_act_single(LAST - 1, st[LAST - 1])
                if i == LAST + 2:
                    stage5_act_single(LAST, st[LAST])

    # Bacc pipeline: splits multi-sem waits (HW allows one per instr),
    # inserts GPSIMD library loads + ACT table loads, lowers extended ISA.
    nc.compile()
    return nc


def _get_nc() -> bass.Bass:
    global _nc_cache
    if _nc_cache is None:
        _nc_cache = _build()
    return _nc_cache


def run(encoder_outputs: np.ndarray, attention_weights: np.ndarray, **spmd_kwargs):
    """Run the SPMD kernel; returns (output [B, H], BassKernelResults)."""
    nc = _get_nc()
    x = np.ascontiguousarray(encoder_outputs, dtype=np.float32)
    w = np.ascontiguousarray(attention_weights, dtype=np.float32)
    assert x.shape == (B, S, H), x.shape
    assert w.shape == (H, 1), w.shape
    in_maps = [
        {
            "encoder_outputs": x[i * B_SHARD : (i + 1) * B_SHARD],
            "attention_weights": w,
        }
        for i in range(N_CORES)
    ]
    res = run_bass_kernel_spmd(nc, in_maps, core_ids=list(range(N_CORES)), **spmd_kwargs)
    out = np.concatenate(
        [res.results[i]["out"] for i in range(N_CORES)], axis=0
    ).astype(np.float32)
    return out, res


def kernel(encoder_outputs: np.ndarray, attention_weights: np.ndarray) -> np.ndarray:
    out, _ = run(encoder_outputs, attention_weights)
    return out
